# revision 15
# baseline (speedup 1.0000x reference)
"""Trainium2 Bass kernel for nn_DockBase (GNN message passing, 3 layers).

Strategy (8 NeuronCores, SPMD, one program + per-core data):
  - Host: sort edges by dst; shard by dst-range so each core owns N/8 nodes
    and all their incoming edges. Segment sums are then fully core-local;
    the only collective is a per-layer AllGather of the gather table.
  - Per layer each core computes u_src = x @ W1_src for its nodes; [u_src |
    pos] rows form a [N, 192] table replicated via AllGather.
  - Edge loop over fixed-shape chunks (512 lo-src + 512 hi-src slots, dst
    window < 128 nodes): dma_gather u/pos rows by src (int16 split table);
    x_dst contribution via expansion matmul (u_dst_chunk as lhsT vs onehotT);
    message MLP runs feature-major; gathered u_src tiles join the h1 PSUM
    via PE transpose-accumulate; segment sums are one-hot matmuls; per-chunk
    results land in DRAM via dma_scatter_add with host-built index tables
    (disjoint windows + dump row), keeping the program core-uniform.
  - Node MLP data-parallel over local 512-node blocks; output = pos.
"""
import sys

sys.path.insert(0, "/opt/trn_rl_repo")

import numpy as np

import concourse.bass as bass
import concourse.mybir as mybir
import concourse.tile as tile
from concourse import bacc
from concourse.bass_utils import run_bass_kernel_spmd

F32 = mybir.dt.float32
I16 = mybir.dt.int16
AF = mybir.ActivationFunctionType
OP = mybir.AluOpType

NCORES = 8
H = 128
CH = 1024          # edge slots per chunk (4 lo subtiles + 4 hi subtiles)
TW = 192           # gather table row width: u(128) | pos(3) | pad
PW = 64            # pos table row width (256B rows for dma_gather)
IW = 192           # intra table row stride (scatter stride must be 256B mult)


# ----------------------------------------------------------------- host prep

def _chunk_core(sdst_loc, ssrc, split):
    """Chunk one core's dst-sorted local edges.

    Chunk = up to 512 lo-src + 512 hi-src edges, dst window < 128 nodes,
    ends on a node boundary. Returns list of (base, span, lo_ids, hi_ids)
    where ids index into this core's sorted edge array.
    """
    E = len(sdst_loc)
    out = []
    i = 0
    while i < E:
        base = int(sdst_loc[i])
        lo_e, hi_e = [], []
        j = i
        while j < E:
            d = int(sdst_loc[j])
            if d - base >= 128:
                break
            k = j
            while k < E and sdst_loc[k] == d:
                k += 1
            run = np.arange(j, k)
            rlo = run[ssrc[run] < split]
            rhi = run[ssrc[run] >= split]
            if len(lo_e) + len(rlo) > 512 or len(hi_e) + len(rhi) > 512:
                break
            lo_e.extend(rlo.tolist())
            hi_e.extend(rhi.tolist())
            j = k
        assert j > i, "single node run exceeds 512+512 chunk capacity"
        span = int(sdst_loc[j - 1]) - base + 1
        out.append((base, span, np.asarray(lo_e, np.int64), np.asarray(hi_e, np.int64)))
        i = j
    return out


def _preprocess(x, edge_index, edge_attr, pos):
    N = x.shape[0]
    E = edge_index.shape[1]
    NPC = N // NCORES
    NPC_PAD = -(-(NPC + 128) // 512) * 512
    split = N // 2
    assert split < 32768 and N - split < 32768

    src = np.asarray(edge_index[0]).astype(np.int64)
    dst = np.asarray(edge_index[1]).astype(np.int64)
    deg = np.bincount(dst, minlength=N).astype(np.float32)
    denominv_full = (1.0 / np.maximum(deg, 1.0)).astype(np.float32)

    order = np.argsort(dst, kind="stable")
    sdst = dst[order]
    ssrc = src[order]
    sea = np.asarray(edge_attr)[order]

    bounds = np.searchsorted(sdst, np.arange(NCORES + 1) * NPC)
    per_core_chunks = []
    for c in range(NCORES):
        lo_b, hi_b = int(bounds[c]), int(bounds[c + 1])
        ch = _chunk_core(sdst[lo_b:hi_b] - c * NPC, ssrc[lo_b:hi_b], split)
        per_core_chunks.append((lo_b, ch))
    NCH = max(len(ch) for _, ch in per_core_chunks)
    DUMP = NPC  # zeroed pad row absorbing scatter padding

    cores = []
    for c in range(NCORES):
        lo_b, chunks = per_core_chunks[c]
        srcidx = np.zeros((16, NCH * 64), dtype=np.int16)
        udidx = np.zeros((16, NCH * 8), dtype=np.int16)
        evidx = np.full((16, NCH * 8), DUMP, dtype=np.int16)
        dstloc = np.full((128, NCH * 8), -1000.0, dtype=np.float32)
        eaT = np.zeros((NCH, 128, CH), dtype=np.float32)
        for ci in range(NCH):
            if ci < len(chunks):
                base, span, lo, hi = chunks[ci]
            else:
                base, span = 0, 0
                lo = hi = np.zeros((0,), np.int64)
            idx_vals = np.zeros(CH, dtype=np.int16)
            dl_vals = np.full(CH, -1000.0, dtype=np.float32)
            if len(lo):
                idx_vals[: len(lo)] = ssrc[lo_b + lo].astype(np.int16)
                dl_vals[: len(lo)] = (sdst[lo_b + lo] - c * NPC - base).astype(np.float32)
                eaT[ci, :, : len(lo)] = sea[lo_b + lo].T
            if len(hi):
                idx_vals[512 : 512 + len(hi)] = (ssrc[lo_b + hi] - split).astype(np.int16)
                dl_vals[512 : 512 + len(hi)] = (sdst[lo_b + hi] - c * NPC - base).astype(np.float32)
                eaT[ci, :, 512 : 512 + len(hi)] = sea[lo_b + hi].T
            srcidx[:, ci * 64 : (ci + 1) * 64] = idx_vals.reshape(64, 16).T
            dstloc[:, ci * 8 : (ci + 1) * 8] = dl_vals.reshape(8, 128).T
            uv = (base + np.arange(128)).astype(np.int16)
            udidx[:, ci * 8 : (ci + 1) * 8] = uv.reshape(8, 16).T
            ev = np.full(128, DUMP, dtype=np.int16)
            ev[:span] = base + np.arange(span)
            evidx[:, ci * 8 : (ci + 1) * 8] = ev.reshape(8, 16).T

        dv = np.zeros(NPC_PAD, dtype=np.float32)
        dv[:NPC] = denominv_full[c * NPC : (c + 1) * NPC]
        dinv = dv.reshape(-1, 128).T.copy()      # [128, NPC_PAD//128]

        xT0 = np.zeros((128, NPC_PAD), dtype=np.float32)
        xT0[:, :NPC] = np.asarray(x)[c * NPC : (c + 1) * NPC].T
        posn0 = np.zeros((NPC_PAD, PW), dtype=np.float32)
        posn0[:NPC, :3] = np.asarray(pos)[c * NPC : (c + 1) * NPC]

        cores.append(
            dict(srcidx=np.tile(srcidx, (8, 1)), udidx=np.tile(udidx, (8, 1)),
                 evidx=np.tile(evidx, (8, 1)), dstloc=dstloc, eaT=eaT,
                 dinv=dinv, xT0=xT0, posn0=posn0)
        )
    dims = dict(N=N, E=E, NPC=NPC, NPC_PAD=NPC_PAD, NBLK=NPC_PAD // 512,
                NCH=NCH, split=split)
    return cores, dims


def _prep_weights(ins):
    W = {}
    mW1 = np.ascontiguousarray(ins["msg_W1"], dtype=np.float32)   # [3,384,128]
    W["msgW1"] = mW1.reshape(3, 3, 128, 128).transpose(0, 2, 1, 3).copy()
    W["msgW2"] = np.ascontiguousarray(ins["msg_W2"], dtype=np.float32)
    W["mb1"] = np.ascontiguousarray(ins["msg_b1"], dtype=np.float32)[:, :, None]
    W["mb2bc"] = np.tile(np.asarray(ins["msg_b2"], dtype=np.float32)[:, None, :], (1, 128, 1))
    aW1 = np.ascontiguousarray(ins["acc_W1"], dtype=np.float32)   # [3,128,256]
    W["accW1"] = aW1.reshape(3, 128, 2, 128).copy()
    ab1p = np.asarray(ins["acc_b1"], dtype=np.float32) + np.einsum(
        "lh,lhk->lk", np.asarray(ins["msg_b2"], dtype=np.float32), aW1)
    W["ab1p"] = ab1p.reshape(3, 2, 128).transpose(0, 2, 1).copy()
    W["accW2"] = np.asarray(ins["acc_W2"], dtype=np.float32).reshape(3, 2, 128).transpose(0, 2, 1).copy()
    W["ab2"] = [float(v) for v in np.asarray(ins["acc_b2"]).reshape(3)]
    nW1 = np.ascontiguousarray(ins["node_W1"], dtype=np.float32)  # [3,256,256]
    W["nW1"] = nW1.reshape(3, 2, 128, 2, 128).transpose(0, 2, 1, 3, 4).reshape(3, 128, 4, 128).copy()
    W["nb1"] = np.asarray(ins["node_b1"], dtype=np.float32).reshape(3, 2, 128).transpose(0, 2, 1).copy()
    nW2 = np.ascontiguousarray(ins["node_W2"], dtype=np.float32)  # [3,256,128]
    W["nW2"] = nW2.reshape(3, 2, 128, 128).transpose(0, 2, 1, 3).copy()
    W["nb2"] = np.asarray(ins["node_b2"], dtype=np.float32)[:, :, None]
    return W


# ------------------------------------------------------------- kernel build

def _build(dims, ab2, n_layers=3):
    N, NPC, NPC_PAD, NBLK, NCH = (
        dims["N"], dims["NPC"], dims["NPC_PAD"], dims["NBLK"], dims["NCH"])
    split = dims["split"]

    import os
    F_NOAG = os.environ.get("K_NOAG") == "1"
    F_NOGATHER = os.environ.get("K_NOGATHER") == "1"
    F_NOSCATTER = os.environ.get("K_NOSCATTER") == "1"
    F_NOEDGE = os.environ.get("K_NOEDGE") == "1"
    F_NONODE = os.environ.get("K_NONODE") == "1"
    F_NOSTAGEA = os.environ.get("K_NOSTAGEA") == "1"
    F_NOBCT = os.environ.get("K_NOBCT") == "1"
    F_NOTRACC = os.environ.get("K_NOTRACC") == "1"
    F_NOW4 = os.environ.get("K_NOW4") == "1"
    F_NOAGG = os.environ.get("K_NOAGG") == "1"
    F_NOPD = os.environ.get("K_NOPD") == "1"
    LVL = int(os.environ.get("K_EDGELVL", "99"))
    F_NOTTR = os.environ.get("K_NOTTR") == "1"
    nc = bacc.Bacc("TRN2", target_bir_lowering=False, debug=False,
                   num_devices=NCORES)

    t_iota_row = nc.dram_tensor("iota_row", [128, 128], F32, kind="ExternalInput")
    t_iota_col = nc.dram_tensor("iota_col", [128, 1], F32, kind="ExternalInput")
    t_ident = nc.dram_tensor("ident", [128, 128], F32, kind="ExternalInput")
    t_srcidx = nc.dram_tensor("srcidx", [128, NCH * 64], I16, kind="ExternalInput")
    t_udidx = nc.dram_tensor("udidx", [128, NCH * 8], I16, kind="ExternalInput")
    t_evidx = nc.dram_tensor("evidx", [128, NCH * 8], I16, kind="ExternalInput")
    t_dstloc = nc.dram_tensor("dstloc", [128, NCH * 8], F32, kind="ExternalInput")
    t_eaT = nc.dram_tensor("eaT", [NCH, 128, CH], F32, kind="ExternalInput")
    t_dinv = nc.dram_tensor("dinv", [128, NPC_PAD // 128], F32, kind="ExternalInput")
    t_xT0 = nc.dram_tensor("xT0", [128, NPC_PAD], F32, kind="ExternalInput")
    t_posn0 = nc.dram_tensor("posn0", [NPC_PAD, PW], F32, kind="ExternalInput")
    t_msgW1 = nc.dram_tensor("msgW1", [3, 128, 3, 128], F32, kind="ExternalInput")
    t_msgW2 = nc.dram_tensor("msgW2", [3, 128, 128], F32, kind="ExternalInput")
    t_mb1 = nc.dram_tensor("mb1", [3, 128, 1], F32, kind="ExternalInput")
    t_mb2bc = nc.dram_tensor("mb2bc", [3, 128, 128], F32, kind="ExternalInput")
    t_accW1 = nc.dram_tensor("accW1", [3, 128, 2, 128], F32, kind="ExternalInput")
    t_ab1p = nc.dram_tensor("ab1p", [3, 128, 2], F32, kind="ExternalInput")
    t_accW2 = nc.dram_tensor("accW2", [3, 128, 2], F32, kind="ExternalInput")
    t_nW1 = nc.dram_tensor("nW1", [3, 128, 4, 128], F32, kind="ExternalInput")
    t_nb1 = nc.dram_tensor("nb1", [3, 128, 2], F32, kind="ExternalInput")
    t_nW2 = nc.dram_tensor("nW2", [3, 128, 2, 128], F32, kind="ExternalInput")
    t_nb2 = nc.dram_tensor("nb2", [3, 128, 1], F32, kind="ExternalInput")

    t_out = nc.dram_tensor("out_pos", [NPC, 4], F32, kind="ExternalOutput")

    d_posn = nc.dram_tensor("posn_d", [NPC_PAD, PW], F32)
    d_udst = nc.dram_tensor("udst_d", [NPC_PAD, 128], F32)
    d_uloc = nc.dram_tensor("uloc_d", [NPC_PAD, TW], F32)
    d_intra = nc.dram_tensor("intra_d", [NPC_PAD, IW], F32)
    import os
    _shared = os.environ.get("K_NOSHARED") != "1"
    d_upos = nc.dram_tensor("upos_sh", [N, TW], F32, addr_space="Shared" if _shared else "Local")

    from contextlib import ExitStack

    with tile.TileContext(nc) as tc, ExitStack() as stack:
        cst = stack.enter_context(tc.tile_pool(name="cst", bufs=1))
        wts = stack.enter_context(tc.tile_pool(name="wts", bufs=1))
        wrk = stack.enter_context(tc.tile_pool(name="wrk", bufs=3))
        ps = stack.enter_context(tc.tile_pool(name="ps", bufs=2, space="PSUM"))

        iota_row = cst.tile([128, 128], F32)
        nc.sync.dma_start(iota_row[:], t_iota_row[:, :])
        iota_col = cst.tile([128, 1], F32)
        nc.sync.dma_start(iota_col[:], t_iota_col[:, :])
        ident = cst.tile([128, 128], F32)
        nc.sync.dma_start(ident[:], t_ident[:, :])
        srcidx = cst.tile([128, NCH * 64], I16)
        nc.sync.dma_start(srcidx[:], t_srcidx[:, :])
        udidx = cst.tile([128, NCH * 8], I16)
        nc.sync.dma_start(udidx[:], t_udidx[:, :])
        evidx = cst.tile([128, NCH * 8], I16)
        nc.sync.dma_start(evidx[:], t_evidx[:, :])
        dstloc = cst.tile([128, NCH * 8], F32)
        nc.sync.dma_start(dstloc[:], t_dstloc[:, :])
        dinv = cst.tile([128, NPC_PAD // 128], F32)
        nc.sync.dma_start(dinv[:], t_dinv[:, :])
        xT = cst.tile([128, NPC_PAD], F32)
        nc.sync.dma_start(xT[:], t_xT0[:, :])
        zline = cst.tile([128, IW], F32)
        nc.vector.memset(zline[:], 0.0)

        # d_posn <- posn0 (device-owned copy; inputs must stay immutable)
        nc.sync.dma_start(d_posn[:, :], t_posn0[:, :])

        for l in range(n_layers):
            last = l == n_layers - 1
            # ---- weights for this layer
            wmsg1 = wts.tile([128, 3, 128], F32, tag="wmsg1")
            nc.sync.dma_start(wmsg1[:], t_msgW1[l])
            wmsg2 = wts.tile([128, 128], F32, tag="wmsg2")
            nc.sync.dma_start(wmsg2[:], t_msgW2[l])
            wmb1 = wts.tile([128, 1], F32, tag="wmb1")
            nc.sync.dma_start(wmb1[:], t_mb1[l])
            wmb2bc = wts.tile([128, 128], F32, tag="wmb2bc")
            nc.sync.dma_start(wmb2bc[:], t_mb2bc[l])
            waccW1 = wts.tile([128, 2, 128], F32, tag="waccW1")
            nc.sync.dma_start(waccW1[:], t_accW1[l])
            wab1p = wts.tile([128, 2], F32, tag="wab1p")
            nc.sync.dma_start(wab1p[:], t_ab1p[l])
            waccW2 = wts.tile([128, 2], F32, tag="waccW2")
            nc.sync.dma_start(waccW2[:], t_accW2[l])
            if not last:
                wnW1 = wts.tile([128, 4, 128], F32, tag="wnW1")
                nc.sync.dma_start(wnW1[:], t_nW1[l])
                wnb1 = wts.tile([128, 2], F32, tag="wnb1")
                nc.sync.dma_start(wnb1[:], t_nb1[l])
                wnW2 = wts.tile([128, 2, 128], F32, tag="wnW2")
                nc.sync.dma_start(wnW2[:], t_nW2[l])
                wnb2 = wts.tile([128, 1], F32, tag="wnb2")
                nc.sync.dma_start(wnb2[:], t_nb2[l])

            # ---- zero the intra table (scatter-add targets)
            for b in range(NPC_PAD // 128):
                nc.sync.dma_start(d_intra[b * 128 : (b + 1) * 128, :], zline[:])

            # ---- stage A: u tables + AllGather
            for b in range(NBLK if not F_NOSTAGEA else 0):
                xb = xT[:, b * 512 : (b + 1) * 512]
                us_ps = ps.tile([128, 512], F32, tag="pbig", bufs=3)
                nc.tensor.matmul(us_ps[:], wmsg1[:, 1], xb, start=True, stop=True)
                uss = wrk.tile([128, 512], F32, tag="uss")
                nc.vector.tensor_copy(uss[:], us_ps[:])
                ud_ps = ps.tile([128, 512], F32, tag="pbig", bufs=3)
                nc.tensor.matmul(ud_ps[:], wmsg1[:, 0], xb, start=True, stop=True)
                uds = wrk.tile([128, 512], F32, tag="uds")
                nc.vector.tensor_copy(uds[:], ud_ps[:])
                pn4 = wrk.tile([128, 4, 4], F32, tag="pn4")
                nc.sync.dma_start(
                    pn4[:],
                    d_posn[b * 512 : (b + 1) * 512, 0:4].rearrange(
                        "(j p) f -> p j f", p=128),
                )
                for j in range(4):
                    tp = ps.tile([128, 128], F32, tag="psmall", bufs=2)
                    nc.tensor.matmul(tp[:], uss[:, j * 128 : (j + 1) * 128], ident[:],
                                     is_transpose=True, start=True, stop=True)
                    stg = wrk.tile([128, 132], F32, tag="stg")
                    nc.vector.tensor_copy(stg[:, 0:128], tp[:])
                    nc.vector.tensor_copy(stg[:, 128:131], pn4[:, j, 0:3])
                    r0 = b * 512 + j * 128
                    nc.sync.dma_start(d_uloc[r0 : r0 + 128, 0:131], stg[:, 0:131])
                    tp2 = ps.tile([128, 128], F32, tag="psmall", bufs=2)
                    nc.tensor.matmul(tp2[:], uds[:, j * 128 : (j + 1) * 128], ident[:],
                                     is_transpose=True, start=True, stop=True)
                    udj = wrk.tile([128, 128], F32, tag="udj")
                    nc.vector.tensor_copy(udj[:], tp2[:])
                    nc.sync.dma_start(d_udst[r0 : r0 + 128, :], udj[:])

            if not F_NOAG:
                nc.gpsimd.collective_compute(
                    "AllGather", OP.bypass,
                    replica_groups=[list(range(NCORES))],
                    ins=[d_uloc[0:NPC, :]],
                    outs=[d_upos[:, :]],
                )

            # ---- stage B: edge loop
            for c in range(NCH if not F_NOEDGE else 0):
                ea_t = wrk.tile([128, CH], F32, tag="ea")
                nc.sync.dma_start(ea_t[:], t_eaT[c])
                upos = wrk.tile([128, 8, TW], F32, tag="upos")
                if not F_NOGATHER:
                    nc.gpsimd.dma_gather(
                        out_ap=upos[:, 0:4, :], in_ap=d_upos[0:split, :],
                        idxs_ap=srcidx[:, c * 64 : c * 64 + 32],
                        num_idxs=512, num_idxs_reg=512, elem_size=TW)
                    nc.gpsimd.dma_gather(
                        out_ap=upos[:, 4:8, :], in_ap=d_upos[split:N, :],
                        idxs_ap=srcidx[:, c * 64 + 32 : c * 64 + 64],
                        num_idxs=512, num_idxs_reg=512, elem_size=TW)
                else:
                    nc.vector.memset(upos[:], 0.25)
                udc = wrk.tile([128, 1, 128], F32, tag="udc")
                pnc = wrk.tile([128, 1, PW], F32, tag="pnc")
                if not F_NOGATHER:
                    nc.gpsimd.dma_gather(
                        out_ap=udc[:], in_ap=d_udst[:, :],
                        idxs_ap=udidx[:, c * 8 : (c + 1) * 8],
                        num_idxs=128, num_idxs_reg=128, elem_size=128)
                    nc.gpsimd.dma_gather(
                        out_ap=pnc[:], in_ap=d_posn[:, :],
                        idxs_ap=udidx[:, c * 8 : (c + 1) * 8],
                        num_idxs=128, num_idxs_reg=128, elem_size=PW)
                else:
                    nc.vector.memset(udc[:], 0.1)
                    nc.vector.memset(pnc[:], 0.2)

                intra_ps = ps.tile([128, 132], F32, tag="intra", bufs=1)
                for h in range(2):
                    ohT = wrk.tile([128, 512], F32, tag="ohT")
                    if F_NOBCT:
                        nc.vector.memset(ohT[:], 0.0)
                    else:
                        for jj in range(4):
                            j = 4 * h + jj
                            bc_ps = ps.tile([128, 128], F32, tag="psmall", bufs=2)
                            nc.tensor.matmul(
                                bc_ps[:],
                                dstloc[:, c * 8 + j : c * 8 + j + 1].to_broadcast([128, 128]),
                                ident[:], is_transpose=True, start=True, stop=True)
                            nc.vector.tensor_tensor(
                                out=ohT[:, jj * 128 : (jj + 1) * 128],
                                in0=iota_col[:, 0:1].to_broadcast([128, 128]),
                                in1=bc_ps[:], op=OP.is_equal)

                    h1_ps = ps.tile([128, 512], F32, tag="pbig", bufs=3)
                    nc.tensor.matmul(h1_ps[:], wmsg1[:, 2],
                                     ea_t[:, h * 512 : (h + 1) * 512],
                                     start=True, stop=False)
                    nc.tensor.matmul(h1_ps[:], udc[:, 0, :], ohT[:],
                                     start=False, stop=False)
                    if F_NOTRACC:
                        nc.tensor.matmul(h1_ps[:], udc[:, 0, :], ohT[:],
                                         start=False, stop=True)
                    else:
                        for jj in range(4):
                            nc.tensor.matmul(
                                h1_ps[:, jj * 128 : (jj + 1) * 128],
                                upos[:, 4 * h + jj, 0:128], ident[:],
                                is_transpose=True, start=False, stop=(jj == 3))
                    h1s = wrk.tile([128, 512], F32, tag="h1s")
                    nc.scalar.activation(h1s[:], h1_ps[:], AF.Relu, bias=wmb1[:, 0:1])
                    if LVL < 2:
                        continue

                    mT_ps = ps.tile([128, 512], F32, tag="pbig", bufs=3)
                    nc.tensor.matmul(mT_ps[:], wmsg2[:], h1s[:], start=True, stop=True)
                    mTs = wrk.tile([128, 512], F32, tag="mTs")
                    nc.vector.tensor_copy(mTs[:], mT_ps[:])

                    if LVL < 3:
                        continue
                    haccs = []
                    for m in range(2):
                        a_ps = ps.tile([128, 512], F32, tag="pbig", bufs=3)
                        nc.tensor.matmul(a_ps[:], waccW1[:, m], mTs[:], start=True, stop=True)
                        hs = wrk.tile([128, 512], F32, tag=f"hacc{m}")
                        nc.scalar.activation(hs[:], a_ps[:], AF.Relu,
                                             bias=wab1p[:, m : m + 1])
                        haccs.append(hs)
                    if LVL < 4:
                        continue
                    w4_ps = ps.tile([128, 4], F32, tag="w4", bufs=1)
                    if F_NOW4:
                        nc.tensor.matmul(w4_ps[:], haccs[0][:, 0:128],
                                         waccW2[:, 0:1].to_broadcast([128, 4]),
                                         start=True, stop=True)
                    else:
                        for jj in range(4):
                            for m in range(2):
                                nc.tensor.matmul(
                                    w4_ps[:, jj : jj + 1],
                                    haccs[m][:, jj * 128 : (jj + 1) * 128],
                                    waccW2[:, m : m + 1],
                                    start=(m == 0), stop=(m == 1))

                    if LVL < 5:
                        continue
                    aggw = 132 if not last else 4
                    aggrhs = [wrk.tile([128, aggw], F32, tag=f"agr{jj}", name=f"agr{jj}") for jj in range(4)]
                    acol = 128 if not last else 0
                    if not last:
                        for jj in range(4):
                            me_ps = ps.tile([128, 128], F32, tag="psmall", bufs=2)
                            nc.tensor.matmul(me_ps[:],
                                             h1s[:, jj * 128 : (jj + 1) * 128],
                                             wmsg2[:], start=True, stop=True)
                            nc.vector.tensor_tensor(out=aggrhs[jj][:, 0:128],
                                                    in0=me_ps[:], in1=wmb2bc[:],
                                                    op=OP.add)
                    if LVL < 6:
                        continue
                    rel_t = wrk.tile([128, 16], F32, tag="rel")
                    nc.vector.memset(rel_t[:], 0.0)
                    d2 = wrk.tile([128, 4], F32, tag="d2")
                    sq16 = wrk.tile([128, 16], F32, tag="sq16")
                    for jj in range(4):
                        pd_ps = ps.tile([128, 4], F32, tag="psmall", bufs=2)
                        nc.tensor.matmul(pd_ps[:],
                                         ohT[:, jj * 128 : (jj + 1) * 128] if not F_NOPD else ohT[:, 0:128],
                                         pnc[:, 0, 0:4], start=True, stop=True)
                        nc.vector.tensor_tensor(
                            out=rel_t[:, jj * 4 : jj * 4 + 3],
                            in0=upos[:, 4 * h + jj, 128:131],
                            in1=pd_ps[:, 0:3], op=OP.subtract)
                    if not F_NOTTR:
                        nc.vector.tensor_tensor(out=sq16[:], in0=rel_t[:], in1=rel_t[:],
                                                op=OP.mult)
                        nc.vector.tensor_reduce(
                            out=d2[:].rearrange("p (j o) -> p j o", o=1),
                            in_=sq16[:].rearrange("p (j f) -> p j f", f=4)[:, :, 0:3],
                            axis=mybir.AxisListType.X, op=OP.add)
                    else:
                        nc.vector.memset(d2[:], 1.0)
                    if LVL < 7:
                        continue
                    dist4 = wrk.tile([128, 4], F32, tag="dist4")
                    nc.scalar.activation(dist4[:], d2[:], AF.Sqrt)
                    inv4 = wrk.tile([128, 4], F32, tag="inv4")
                    nc.vector.reciprocal(inv4[:], dist4[:])
                    wrs4 = wrk.tile([128, 4], F32, tag="wrs4")
                    nc.vector.tensor_scalar(out=wrs4[:], in0=w4_ps[:],
                                            scalar1=ab2[l], scalar2=None, op0=OP.add)
                    nc.vector.tensor_tensor(out=wrs4[:], in0=wrs4[:], in1=inv4[:],
                                            op=OP.mult)
                    for jj in range(4):
                        nc.vector.tensor_scalar(
                            out=aggrhs[jj][:, acol : acol + 3],
                            in0=rel_t[:, jj * 4 : jj * 4 + 3],
                            scalar1=wrs4[:, jj : jj + 1], scalar2=None, op0=OP.mult)
                    if LVL < 8:
                        continue
                    for jj in range(4):
                        j = 4 * h + jj
                        oh = wrk.tile([128, 128], F32, tag="oh")
                        nc.vector.tensor_tensor(
                            out=oh[:],
                            in0=dstloc[:, c * 8 + j : c * 8 + j + 1].to_broadcast([128, 128]),
                            in1=iota_row[:], op=OP.is_equal)
                        if not F_NOAGG:
                            nc.tensor.matmul(
                                intra_ps[:, 0 : acol + 3], oh[:],
                                aggrhs[jj][:, 0 : acol + 3],
                                start=(h == 0 and jj == 0), stop=(h == 1 and jj == 3))

                # evacuate chunk psum -> scatter into intra table
                if LVL < 8:
                    nc.vector.memset(intra_ps[:], 0.0)
                if F_NOAGG:
                    nc.vector.memset(intra_ps[:], 0.0)
                if not last:
                    ev = wrk.tile([128, 1, 131], F32, tag="ev")
                    nc.vector.tensor_copy(ev[:, 0, :], intra_ps[:, 0:131])
                    if not F_NOSCATTER:
                        nc.gpsimd.dma_scatter_add(
                            out_ap=d_intra[:, 0:131], in_ap=ev[:],
                            idxs_ap=evidx[:, c * 8 : (c + 1) * 8],
                            num_idxs=128, num_idxs_reg=128, elem_size=131, elem_step=IW)
                else:
                    ev3 = wrk.tile([128, 1, 3], F32, tag="ev")
                    nc.vector.tensor_copy(ev3[:, 0, :], intra_ps[:, 0:3])
                    if not F_NOSCATTER:
                        nc.gpsimd.dma_scatter_add(
                            out_ap=d_intra[:, 128:131], in_ap=ev3[:],
                            idxs_ap=evidx[:, c * 8 : (c + 1) * 8],
                            num_idxs=128, num_idxs_reg=128, elem_size=3, elem_step=IW)

            # ---- stage C: node update
            for b in range(NBLK):
                it4 = wrk.tile([128, 4, 132], F32, tag="it4")
                nc.sync.dma_start(
                    it4[:],
                    d_intra[b * 512 : (b + 1) * 512, 0:132].rearrange(
                        "(j p) f -> p j f", p=128))
                pn4 = wrk.tile([128, 4, 4], F32, tag="pn4")
                nc.sync.dma_start(
                    pn4[:],
                    d_posn[b * 512 : (b + 1) * 512, 0:4].rearrange(
                        "(j p) f -> p j f", p=128))
                pnt = wrk.tile([128, 4, 4], F32, tag="pnt")
                iscs = []
                for j in range(4):
                    isc = wrk.tile([128, 132], F32, tag=f"isc{j}")
                    nc.vector.tensor_scalar(
                        out=isc[:], in0=it4[:, j, :],
                        scalar1=dinv[:, b * 4 + j : b * 4 + j + 1],
                        scalar2=None, op0=OP.mult)
                    iscs.append(isc)
                    nc.vector.tensor_tensor(
                        out=pnt[:, j, 0:4], in0=pn4[:, j, 0:4],
                        in1=isc[:, 128:132], op=OP.add)
                if not last:
                    nc.sync.dma_start(
                        d_posn[b * 512 : (b + 1) * 512, 0:4].rearrange(
                            "(j p) f -> p j f", p=128), pnt[:])
                else:
                    for j in range(4):
                        r0 = b * 512 + j * 128
                        if r0 >= NPC:
                            break
                        rows = min(128, NPC - r0)
                        nc.sync.dma_start(t_out[r0 : r0 + rows, :],
                                          pnt[0:rows, j, 0:4])
                if last or F_NONODE:
                    continue
                intraT = wrk.tile([128, 512], F32, tag="intraT")
                for j in range(4):
                    tp = ps.tile([128, 128], F32, tag="psmall", bufs=2)
                    nc.tensor.matmul(tp[:], iscs[j][:, 0:128], ident[:],
                                     is_transpose=True, start=True, stop=True)
                    nc.vector.tensor_copy(intraT[:, j * 128 : (j + 1) * 128], tp[:])
                xb = xT[:, b * 512 : (b + 1) * 512]
                hns = []
                for m in range(2):
                    hn_ps = ps.tile([128, 512], F32, tag="pbig", bufs=3)
                    nc.tensor.matmul(hn_ps[:], wnW1[:, 0 * 2 + m], xb,
                                     start=True, stop=False)
                    nc.tensor.matmul(hn_ps[:], wnW1[:, 1 * 2 + m], intraT[:],
                                     start=False, stop=True)
                    hs = wrk.tile([128, 512], F32, tag=f"hns{m}")
                    nc.scalar.activation(hs[:], hn_ps[:], AF.Relu,
                                         bias=wnb1[:, m : m + 1])
                    hns.append(hs)
                xn_ps = ps.tile([128, 512], F32, tag="pbig", bufs=3)
                nc.tensor.matmul(xn_ps[:], wnW2[:, 0], hns[0][:], start=True, stop=False)
                nc.tensor.matmul(xn_ps[:], wnW2[:, 1], hns[1][:], start=False, stop=True)
                nc.vector.tensor_scalar(out=xb, in0=xn_ps[:], scalar1=wnb2[:, 0:1],
                                        scalar2=None, op0=OP.add)

    nc.compile()
    return nc


# ---------------------------------------------------------------- entrypoint

def kernel(**inputs):
    x = np.asarray(inputs["x"], dtype=np.float32)
    edge_index = np.asarray(inputs["edge_index"])
    edge_attr = np.asarray(inputs["edge_attr"], dtype=np.float32)
    pos = np.asarray(inputs["pos"], dtype=np.float32)

    cores, dims = _preprocess(x, edge_index, edge_attr, pos)
    W = _prep_weights(inputs)
    nc = _build(dims, W["ab2"])

    iota_row = np.tile(np.arange(128, dtype=np.float32)[None, :], (128, 1))
    iota_col = np.arange(128, dtype=np.float32)[:, None]
    ident = np.eye(128, dtype=np.float32)

    in_maps = []
    for c in range(NCORES):
        cc = cores[c]
        m = dict(iota_row=iota_row, iota_col=iota_col, ident=ident,
                 srcidx=cc["srcidx"], udidx=cc["udidx"], evidx=cc["evidx"],
                 dstloc=cc["dstloc"], eaT=cc["eaT"], dinv=cc["dinv"],
                 xT0=cc["xT0"], posn0=cc["posn0"],
                 msgW1=W["msgW1"], msgW2=W["msgW2"], mb1=W["mb1"],
                 mb2bc=W["mb2bc"], accW1=W["accW1"], ab1p=W["ab1p"],
                 accW2=W["accW2"], nW1=W["nW1"], nb1=W["nb1"],
                 nW2=W["nW2"], nb2=W["nb2"])
        in_maps.append(m)

    res = run_bass_kernel_spmd(nc, in_maps, core_ids=list(range(NCORES)))
    NPC = dims["NPC"]
    out = np.concatenate([res.results[c]["out_pos"][:, :3] for c in range(NCORES)], axis=0)
    return out.astype(np.float32)
